# revision 37
# baseline (speedup 1.0000x reference)
"""Causal multi-head attention (B=2, T=2048, D=1024, H=16) on 8 TRN2 NeuronCores.

Sharding: core c = (batch b = c//4, head-group g = c%4). Each core owns 4 heads
(= 256 contiguous dims of D) of one batch: Megatron-style tensor parallelism on
heads x data parallelism on batch.

Design (single fully-pipelined phase; everything but span 0's attention is
emitted as "fillers" interleaved into the attention kt loops so the in-order
PE queue has no phase boundaries and never head-of-line blocks):
  - Out-projection reduction via per-q-span 8-way bf16 AllToAll of the
    normalized attention output yT (rank r's territory = 64-col q-block r of
    each 512-q span, for BOTH batches -- SPMD-uniform, no junk shards). Each
    core then out-projects its territory with the full Wo.
  - Score matmuls pack the two heads of an mc-chunk as two concurrent K=64
    row-group tiles into one [128, 2*512] 2-bank PSUM tile; ONE exp per pair.
    The kt loop software-pipelines scores kt+1 ahead of the AV matmuls of kt;
    fillers pop BEFORE the exp-gated AV matmuls so the in-order PE queue keeps
    running independent work while ACT produces exp(kt). ACT runs ONLY exp --
    a single table set, loaded once at warmup, zero mid-run table switches.
  - AV uses the v_aug 65th-column trick (denominator accumulates as row 64);
    1/den via the single-instruction DVE approx reciprocal (~18 bits, plenty
    upstream of bf16); normalization = PE rank-1 broadcast of 1/den +
    in-place DVE multiply, one span behind attention.
  - Inputs use pre-chunked partition-major DRAM layouts (host-side prep) so
    each tensor loads as a handful of DMAs with 2-16KB per-partition packets
    (the DMA engines are packet-rate-bound below ~2KB): wq + x-first-half
    land ~4us after the queues arm and span-0 projections start immediately;
    a short PE warmup stream covers the arm window (HAM un-throttle). After
    the lead, bulk loads ride sync/gpsimd only -- NEVER the scalar queue,
    whose instruction stream must stay free for the exp activations.
  - Span 0/1/2 normalizes + A2A triggers fire directly at span end (the
    collective stream can back up by tens of us in bad runs; every exchange
    gets a full span of slack before its out-projection consumers).
  - Out-projection: pair 0 = spans 0+1 stacked (M=128), b=0 units spread
    over span 3's kt loop as fillers, b=1 units deferred into the A2A(3)
    flight. Spans 2 and 3 use per-(b,ns) M=64 units: span 2's depend only on
    A2A(2) and fill the A2A(3) flight (with a short dummy-matmul bridge to
    keep HAM warm); span 3's follow the single full-span A2A(3) that fires
    right after the last normalize. bo folds in on DVE at PSUM evacuation
    (or via a rank-1 seed + ACT copy on alternating tail units so the final
    evacuations run on two engines). Output DMAs alternate gpsimd/sync so
    they never queue behind gather DMAs.

Dtypes: all matmul operands bf16 with fp32 PSUM accumulation; softmax exp(s)
without row-max (scores O(1), scale folded into Wq host-side).
"""

import os
import numpy as np
import ml_dtypes

BF16 = ml_dtypes.bfloat16

B, T, D, H = 2, 2048, 1024, 16
HD = D // H                     # 64
NCORES = 8
GROUPS = 4                      # cores per batch (tensor-parallel degree)
HL = H // GROUPS                # heads per core = 4
DL = D // GROUPS                # dims per core = 256
SP = 512                        # free-dim span per matmul (one PSUM bank, fp32)
QS = T // SP                    # 4 q spans
KT = T // 128                   # 16 k tiles
QB = 64                         # q columns per rank territory per span
SCALE = HD ** -0.5

_CACHE = {}


def _build_program():
    import concourse.bass as bass  # noqa: F401  (registers bass machinery)
    import concourse.tile as tile
    from concourse import bacc, mybir

    f32 = mybir.dt.float32
    bf16 = mybir.dt.bfloat16
    Exp = mybir.ActivationFunctionType.Exp

    nc = bacc.Bacc("TRN2", target_bir_lowering=False, debug=False,
                   num_devices=NCORES)

    # pre-chunked host-side layouts: partition-major so each tensor loads
    # as a handful of DMAs whose per-partition runs are 2-16KB (the DMA
    # engines are packet-rate-bound below ~2KB)
    xP = nc.dram_tensor("xP", [128, 2, 8, 2 * SP], bf16, kind="ExternalInput")
    wqP = nc.dram_tensor("wqP", [128, 8, DL], bf16, kind="ExternalInput")
    wkP = nc.dram_tensor("wkP", [128, 8, DL], bf16, kind="ExternalInput")
    wvP = nc.dram_tensor("wvP", [128, 8, DL], bf16, kind="ExternalInput")
    woP = nc.dram_tensor("woP", [128, 8, D], bf16, kind="ExternalInput")
    bqP = nc.dram_tensor("bqP", [128, 2], f32, kind="ExternalInput")
    bkP = nc.dram_tensor("bkP", [128, 2], f32, kind="ExternalInput")
    bv = nc.dram_tensor("bv", [1, DL], bf16, kind="ExternalInput")
    bo = nc.dram_tensor("bo", [1, D], f32, kind="ExternalInput")
    maskd = nc.dram_tensor("maskd", [128, 128], bf16, kind="ExternalInput")
    onesb = nc.dram_tensor("onesb", [1, 128], bf16, kind="ExternalInput")
    out_ext = nc.dram_tensor("out", [QS, 128, D], bf16,
                             kind="ExternalOutput")

    ALL8 = [[0, 1, 2, 3, 4, 5, 6, 7]]

    with tile.TileContext(nc) as tc:
        with tc.tile_pool(name="main", bufs=1) as main, \
             tc.tile_pool(name="dram", bufs=1, space="DRAM") as dram:
            qT_s = main.tile([128, 2, T], bf16)
            kT_s = main.tile([128, 2, T], bf16)
            v_s = main.tile([128, KT, HL * 65], bf16)
            yT_s = main.tile([128, 2, T], bf16)
            woT_s = main.tile([128, 8, D], bf16)
            bq_s = main.tile([128, 2], f32)
            bk_s = main.tile([128, 2], f32)
            bo_bc = main.tile([128, D], f32)
            onesb_s = main.tile([128, 128], bf16)
            bv_bc = main.tile([128, DL], bf16)
            maskd_s = main.tile([128, 128], bf16)
            warm_s = main.tile([128, 2], f32)
            bo_row = main.tile([1, D], bf16)
            warm_sb = main.tile([128, SP], bf16)

            # per-span A2A staging (separate tiles avoid false DRAM deps)
            a2a_in = [dram.tile([8, DL, QB], bf16, name=f"a2ain{i}")
                      for i in range(QS)]
            a2a_out = [dram.tile([8, DL, QB], bf16, name=f"a2aout{i}")
                       for i in range(QS)]

            # PE warmup: back-to-back matmuls on scratch data while the first
            # input DMAs stream in (HAM un-throttle); real projection matmuls
            # take over as soon as their kc operands land
            nc.vector.memset(warm_sb, 1.0)
            with tc.tile_pool(name="warm_psum", bufs=1,
                              space="PSUM") as warm_psum:
                wps = warm_psum.tile([128, SP], f32, tag="w")
                for _ in range(34):
                    nc.tensor.matmul(wps, lhsT=warm_sb[:, 0:128],
                                     rhs=warm_sb, start=True, stop=True)

            # tiny high-priority loads on the sync queue
            nc.sync.dma_start(out=bq_s, in_=bqP[:])
            nc.sync.dma_start(out=bk_s, in_=bkP[:])
            # pre-load the ACT Exp table (the only set the kernel ever uses)
            nc.scalar.activation(warm_s, warm_sb[:, 0:2], Exp)
            # small loads on the scalar queue (bo/onesb wait until after
            # the lead x/wq loads -- they are not needed until much later)
            nc.scalar.dma_start(out=bv_bc, in_=bv[:].to_broadcast([128, DL]))
            # ones column at index 64 of each head's 65-wide block of v_aug
            nc.vector.memset(v_s, 1.0)

            # ---------------- input loads ----------------
            xt_h = main.tile([128, 2, 8, 2 * SP], bf16)
            wq_s = main.tile([128, 8, DL], bf16)
            wk_s = main.tile([128, 8, DL], bf16)
            wv_s = main.tile([128, 8, DL], bf16)

            # lead: wq + x first half, balanced by measured queue speed
            # (gpsimd ~2x faster than sync, scalar in between)
            nc.sync.dma_start(out=wq_s[:], in_=wqP[:])
            nc.gpsimd.dma_start(out=xt_h[:, 0, 0:4, :], in_=xP[:, 0, 0:4, :])
            nc.scalar.dma_start(out=xt_h[:, 0, 4:7, :], in_=xP[:, 0, 4:7, :])
            nc.sync.dma_start(out=xt_h[:, 0, 7:8, :], in_=xP[:, 0, 7:8, :])
            # mid: wk/wv + small tiles (scalar queue stays free for the exp
            # stream from here on)
            nc.gpsimd.dma_start(out=wk_s[:], in_=wkP[:])
            nc.scalar.dma_start(out=wv_s[:], in_=wvP[:])
            nc.sync.dma_start(out=maskd_s, in_=maskd[:])
            nc.sync.dma_start(out=onesb_s,
                              in_=onesb[:].to_broadcast([128, 128]))
            nc.sync.dma_start(out=bo_bc, in_=bo[:].to_broadcast([128, D]))
            # tail loads: x second half + woT on sync/gpsimd
            nc.gpsimd.dma_start(out=xt_h[:, 1, 0:5, :], in_=xP[:, 1, 0:5, :])
            nc.sync.dma_start(out=xt_h[:, 1, 5:8, :], in_=xP[:, 1, 5:8, :])
            nc.gpsimd.dma_start(out=woT_s[:, 0:5, :], in_=woP[:, 0:5, :])
            nc.sync.dma_start(out=woT_s[:, 5:8, :], in_=woP[:, 5:8, :])

            # bf16 copy of bo for the rank-1 out-projection seeds
            nc.vector.tensor_copy(bo_row, bo_bc[0:1, :])


            # ---- single pipelined phase ----
            with tc.tile_pool(name="attn_t", bufs=4) as attn_t, \
                 tc.tile_pool(name="nrm", bufs=2) as nrm, \
                 tc.tile_pool(name="op_sb", bufs=4) as op_sb, \
                 tc.tile_pool(name="sc_psum", bufs=2, space="PSUM") as sc_psum, \
                 tc.tile_pool(name="av_psum", bufs=2, space="PSUM") as av_psum, \
                 tc.tile_pool(name="op_psum", bufs=2, space="PSUM") as op_psum:

                def attention_span(qs, fillers, self_norm=False):
                    # denominator rows at partitions 0/32/64/96 (engine APs
                    # must start 32-aligned); memset keeps unused rows finite
                    den_stack = nrm.tile([97, SP], f32, tag="den")
                    nc.vector.memset(den_stack, 1.0)
                    rec32 = nrm.tile([97, SP], f32, tag="rec32")
                    rec_bf = nrm.tile([97, SP], bf16, tag="recf")
                    nkt = 4 * qs + 4  # causal: later k tiles are all-masked
                    span = slice(qs * SP, (qs + 1) * SP)
                    # carry pre-load: pop ~2 fillers in the first steps of
                    # the span, where the PE would otherwise micro-idle
                    # waiting on exp(0) and HAM would re-throttle
                    pace = {"left": 2 * nkt, "carry": 2.0}

                    def pop_fillers():
                        frac = pace["carry"] + len(fillers) / pace["left"]
                        n_pop = int(frac)
                        pace["carry"] = frac - n_pop
                        pace["left"] -= 1
                        for _ in range(min(n_pop, len(fillers))):
                            fillers.pop(0)()

                    for p in range(2):  # head pair = mc chunk p
                        qa = qT_s[0:64, p, span]
                        qb = qT_s[64:128, p, span]
                        ya = av_psum.tile([65, SP], f32, tag="av")
                        yb = av_psum.tile([65, SP], f32, tag="av")

                        def sc_pair(kt):
                            # diagonal tiles: q columns < 128*(kt-4qs) are
                            # fully masked; stream only the valid sub-range
                            j0 = max(0, (kt - 4 * qs) * 128)
                            scp = sc_psum.tile([128, 2 * SP], f32, tag="sc")
                            nc.tensor.matmul(
                                scp[:, j0:SP],
                                lhsT=kT_s[0:64, p, kt * 128:(kt + 1) * 128],
                                rhs=qa[:, j0:SP], start=True, stop=True)
                            nc.tensor.matmul(
                                scp[:, SP + j0:2 * SP],
                                lhsT=kT_s[64:128, p, kt * 128:(kt + 1) * 128],
                                rhs=qb[:, j0:SP], start=True, stop=True)
                            return scp

                        # software pipeline: scores kt+1 are emitted before
                        # the AV matmuls of kt so the in-order PE queue keeps
                        # feeding ACT while AV waits on exp kt
                        scp = sc_pair(0)
                        for kt in range(nkt):
                            atp = attn_t.tile([128, 2 * SP], bf16, tag="at")
                            j0e = max(0, (kt - 4 * qs) * 128)
                            if j0e >= 256:
                                # mostly-masked diagonal tile: two narrow
                                # exps over the valid ranges beat one full-
                                # width one
                                nc.scalar.activation(atp[:, j0e:SP],
                                                     scp[:, j0e:SP], Exp)
                                nc.scalar.activation(
                                    atp[:, SP + j0e:2 * SP],
                                    scp[:, SP + j0e:2 * SP], Exp)
                            else:
                                nc.scalar.activation(atp, scp, Exp)
                            if kt >= 4 * qs:
                                # diagonal tile: only its 128-col diagonal
                                # block needs masking and that block is the
                                # same tril(128) for every tile
                                jm = (kt - 4 * qs) * 128
                                nc.vector.tensor_mul(
                                    atp[:, jm:jm + 128],
                                    atp[:, jm:jm + 128], maskd_s)
                                nc.vector.tensor_mul(
                                    atp[:, SP + jm:SP + jm + 128],
                                    atp[:, SP + jm:SP + jm + 128], maskd_s)
                            if kt + 1 < nkt:
                                scp = sc_pair(kt + 1)
                            # independent fillers run while ACT produces
                            # exp(kt); they must precede the exp-gated AVs
                            pop_fillers()
                            j0 = max(0, (kt - 4 * qs) * 128)
                            nc.tensor.matmul(
                                ya[:, j0:SP],
                                lhsT=v_s[:, kt, (2 * p) * 65:
                                             (2 * p + 1) * 65],
                                rhs=atp[:, j0:SP],
                                start=(kt == 0), stop=(kt == nkt - 1))
                            nc.tensor.matmul(
                                yb[:, j0:SP],
                                lhsT=v_s[:, kt, (2 * p + 1) * 65:
                                             (2 * p + 2) * 65],
                                rhs=atp[:, SP + j0:2 * SP],
                                start=(kt == 0), stop=(kt == nkt - 1))
                        # evacuate unnormalized yT' + denominators so the
                        # PSUM banks free for the next pair; the last pair's
                        # evacuation is on the A2A(3)-trigger critical path,
                        # so split it across DVE and the idle ACT engine
                        if self_norm and p == 1:
                            # denominators first on DVE (they gate the
                            # reciprocal -> normalize -> A2A(3) trigger);
                            # the y copies ride the idle ACT engine
                            nc.vector.tensor_copy(
                                den_stack[64 * p:64 * p + 1, :], ya[64:65, :])
                            nc.vector.tensor_copy(
                                den_stack[64 * p + 32:64 * p + 33, :],
                                yb[64:65, :])
                            nc.scalar.copy(yT_s[0:64, p, span], ya[0:64, :])
                            nc.scalar.copy(yT_s[64:128, p, span],
                                           yb[0:64, :])
                        else:
                            # ya's copies first: the next pair's first AV
                            # reuses ya's PSUM slot and shouldn't wait for
                            # yb's evacuation too
                            nc.vector.tensor_copy(yT_s[0:64, p, span],
                                                  ya[0:64, :])
                            nc.vector.tensor_copy(
                                den_stack[64 * p:64 * p + 1, :], ya[64:65, :])
                            nc.vector.tensor_copy(yT_s[64:128, p, span],
                                                  yb[0:64, :])
                            nc.vector.tensor_copy(
                                den_stack[64 * p + 32:64 * p + 33, :],
                                yb[64:65, :])
                        if self_norm:
                            # last span: full-width DVE reciprocal per pair
                            # (the custom DVE op wants partition offset 0;
                            # pair-1 rows are memset 1.0 until valid and the
                            # second pass overwrites everything)
                            nc.vector.reciprocal_approx_fast(
                                out=rec32, in_=den_stack)
                            nc.vector.tensor_copy(rec_bf, rec32)
                            if p == 0:
                                # front-insert: the mc0 normalize must fire
                                # early in pair 1's sweep
                                fillers[0:0] = [norm_h(qs, rec_bf, 0),
                                                norm_h(qs, rec_bf, 1)]
                                pace["carry"] += 2.0
                    if not self_norm:
                        # single-instruction DVE reciprocal: no ACT table
                        # switches, no span-boundary exp contention
                        nc.vector.reciprocal_approx_fast(out=rec32,
                                                         in_=den_stack)
                        nc.vector.tensor_copy(rec_bf, rec32)
                    return rec_bf

                def norm_h(qs, rec_bf, h):
                    def f():
                        span = slice(qs * SP, (qs + 1) * SP)
                        mc, r0 = divmod(h, 2)
                        r0 *= 64
                        rb = op_psum.tile([64, SP], f32, tag="op")
                        r0p = 32 * h
                        nc.tensor.matmul(rb,
                                         lhsT=onesb_s[r0p:r0p + 1, 0:64],
                                         rhs=rec_bf[r0p:r0p + 1, :],
                                         start=True, stop=True,
                                         tile_position=(r0p, 0))
                        nc.vector.tensor_mul(yT_s[r0:r0 + 64, mc, span],
                                             yT_s[r0:r0 + 64, mc, span],
                                             rb)
                    return f

                def stage_a2a(qs):
                    def f():
                        span = slice(qs * SP, (qs + 1) * SP)
                        in_r = a2a_in[qs][:].rearrange(
                            "j (two p) q -> two p j q", p=128)
                        # the staging fragments into 128B packets (per-rank
                        # chunks split every partition row): run the two mc
                        # halves on different queues so the trigger waits
                        # half as long
                        for mc, q in ((0, nc.sync), (1, nc.gpsimd)):
                            q.dma_start(
                                out=in_r[mc],
                                in_=yT_s[:, mc, span].rearrange(
                                    "p (j q) -> p j q", q=QB))
                        nc.gpsimd.collective_compute(
                            "AllToAll", mybir.AluOpType.bypass,
                            replica_groups=ALL8,
                            ins=[a2a_in[qs][:].opt()],
                            outs=[a2a_out[qs][:].opt()])
                    return f

                def proj_qk(w_s, b_s, dst, mc, s):
                    # qT/kT group: out[dims-chunk mc, t-span s]; bias added
                    # on DVE during the PSUM->SBUF evacuation
                    def f():
                        ps = op_psum.tile([128, SP], f32, tag="op",
                                          name="pj")
                        t0 = (s % 2) * SP
                        for kc in range(8):
                            nc.tensor.matmul(
                                ps,
                                lhsT=w_s[:, kc, mc * 128:(mc + 1) * 128],
                                rhs=xt_h[:, s // 2, kc, t0:t0 + SP],
                                start=(kc == 0), stop=(kc == 7))
                        nc.vector.tensor_scalar_add(
                            dst[:, mc, s * SP:(s + 1) * SP],
                            ps, b_s[:, mc:mc + 1])
                    return f

                def proj_v(mt):
                    # v tile in natural [t, d] layout; bias via DVE add into
                    # the 65-stride v_aug slots
                    def f():
                        ps = op_psum.tile([128, SP], f32, tag="op",
                                          name="pjv")
                        t0 = (mt % 8) * 128
                        for kc in range(8):
                            nc.tensor.matmul(
                                ps[:, 0:DL],
                                lhsT=xt_h[:, mt // 8, kc, t0:t0 + 128],
                                rhs=wv_s[:, kc, :],
                                start=(kc == 0), stop=(kc == 7))
                        nc.vector.tensor_add(
                            v_s[:, mt, :].rearrange(
                                "p (h d) -> p h d", d=65)[:, :, 0:64],
                            ps[:, 0:DL].rearrange("p (h d) -> p h d", d=64),
                            bv_bc.rearrange("p (h d) -> p h d", d=64))
                    return f

                def proj_span_fillers(s):
                    fs = []
                    for mc in range(2):
                        fs.append(proj_qk(wq_s, bq_s, qT_s, mc, s))
                    for mc in range(2):
                        fs.append(proj_qk(wk_s, bk_s, kT_s, mc, s))
                    for mt in range(4 * s, 4 * s + 4):
                        fs.append(proj_v(mt))
                    return fs

                def outproj_pair0_fillers():
                    """Out-projection for spans (0,1) with M=128 (both spans'
                    64-q territories stacked), split into per-(b,ns) filler
                    chunks; spread over span 3's kt loop."""
                    yg = op_sb.tile([128, 8, 2, 2 * QB], bf16, tag="yg",
                                    bufs=1)

                    def gather(sp):
                        def f():
                            out_r = a2a_out[sp][:].rearrange(
                                "(b j2) (h p) q -> b p (j2 h) q", j2=4, p=128)
                            m = sp % 2
                            for b in range(2):
                                nc.sync.dma_start(
                                    out=yg[:, :, b, m * QB:(m + 1) * QB],
                                    in_=out_r[b])
                        return f

                    def po_mms(b, ns, kcs, box, last=False):
                        def f():
                            if not box:
                                box.append(op_psum.tile(
                                    [128, SP], f32, tag="op", name="po"))
                            po = box[0]
                            for kc in kcs:
                                nc.tensor.matmul(
                                    po, lhsT=yg[:, kc, b, :],
                                    rhs=woT_s[:, kc, ns * SP:(ns + 1) * SP],
                                    start=(kc == 0),
                                    stop=(last and kc == kcs[-1]))
                            if last:
                                ob = op_sb.tile([128, SP], bf16, tag="ob")
                                nc.vector.tensor_add(
                                    ob, po, bo_bc[:, ns * SP:(ns + 1) * SP])
                                for m in range(2):
                                    nc.gpsimd.dma_start(
                                        out=out_ext[m, b * 64:(b + 1) * 64,
                                                    ns * SP:(ns + 1) * SP],
                                        in_=ob[m * 64:(m + 1) * 64, :])
                        return f

                    fs = [gather(0), gather(1)]
                    for b in range(2):
                        for ns in range(2):
                            box = []
                            fs.append(po_mms(b, ns, [0, 1, 2], box))
                            fs.append(po_mms(b, ns, [3, 4, 5], box))
                            fs.append(po_mms(b, ns, [6, 7], box, last=True))
                    return fs

                def gather_span(sp, tag):
                    yg = op_sb.tile([128, 8, 2, QB], bf16, tag=tag, bufs=1,
                                    name=tag)
                    out_r = a2a_out[sp][:].rearrange(
                        "(b j2) (h p) q -> b p (j2 h) q", j2=4, p=128)
                    for b in range(2):
                        nc.sync.dma_start(out=yg[:, :, b, :], in_=out_r[b])
                    return yg

                def po_unit(sp, yg, b, ns, alt=False):
                    # M=64 out-projection of span sp's territory; engines
                    # alternate so the teardown isn't single-queue-bound.
                    def f():
                        po = op_psum.tile([64, SP], f32, tag="op", name="pou")
                        if alt:
                            # fold bo via a rank-1 seed so the evacuation is
                            # a plain copy the otherwise-idle ACT engine can
                            # run in parallel with DVE's add-evacuations
                            nc.tensor.matmul(
                                po, lhsT=onesb_s[0:1, 0:64],
                                rhs=bo_row[0:1, ns * SP:(ns + 1) * SP],
                                start=True, stop=False)
                        for kc in range(8):
                            nc.tensor.matmul(
                                po, lhsT=yg[:, kc, b, :],
                                rhs=woT_s[:, kc, ns * SP:(ns + 1) * SP],
                                start=(not alt and kc == 0),
                                stop=(kc == 7))
                        ob = op_sb.tile([64, SP], bf16, tag="ob")
                        if alt:
                            nc.scalar.copy(ob, po)
                        else:
                            nc.vector.tensor_add(
                                ob, po, bo_bc[0:64, ns * SP:(ns + 1) * SP])
                        dq = nc.sync if alt else nc.gpsimd
                        dq.dma_start(
                            out=out_ext[sp, b * 64:(b + 1) * 64,
                                        ns * SP:(ns + 1) * SP],
                            in_=ob)
                    return f

                # prologue: only q-mc0/k-mc0 (all the first score matmul
                # needs) run inline; every v unit rides the kt loop as an
                # early filler so the in-order PE queue never stalls on the
                # later-arriving wv load (the pacing front-bias guarantees
                # v(kt) pops before AV(kt))
                p0 = proj_span_fillers(0)
                for f in [p0[0], p0[2]]:                # q0, k0
                    f()
                pend = [p0[4], p0[5], p0[6], p0[7],     # v0..v3
                        p0[1], p0[3]]                   # q1, k1
                rec = {}
                for qs in range(QS):
                    if qs < 3:
                        # span qs+1's projections drain during span qs
                        pend.extend(proj_span_fillers(qs + 1))
                    else:
                        # A2A(0)/(1) completed spans ago: pair-0's b=0
                        # out-projection spreads over span 3's kt loop; the
                        # b=1 units are deferred into the A2A(3) flight
                        opf = outproj_pair0_fillers()
                        pend.extend(opf[:8])

                        def dummy_filler():
                            def f():
                                dp = op_psum.tile([128, SP], f32, tag="op",
                                                  name="dumf")
                                nc.tensor.matmul(dp, lhsT=warm_sb[:, 0:128],
                                                 rhs=warm_sb, start=True,
                                                 stop=True)
                            return f
                        # span 3's late kt steps are ACT-bound with the
                        # filler list nearly drained: pad with dummy matmuls
                        # so the PE never micro-idles (HAM stays warm)
                        pend.extend(dummy_filler() for _ in range(6))
                    if qs == 3:
                        yg2 = gather_span(2, "yg2")
                    rec[qs] = attention_span(qs, pend, self_norm=(qs == 3))
                    if qs < 3:
                        # normalize + trigger the exchange right at span end:
                        # the collective stream can back up by tens of us in
                        # bad runs, and every A2A must land well before its
                        # out-projection consumers
                        for h in range(HL):
                            norm_h(qs, rec[qs], h)()
                        stage_a2a(qs)()
                for f in pend:
                    f()

                # ---- tail: normalize span-3 mc1, fire the single full-span
                # A2A(3), then fill its ~20us flight with span-2's
                # out-projection, pair-0's deferred b=1 units, and a short
                # dummy-matmul bridge (keeps HAM at full clock so the
                # A2A-gated span-3 out-projection runs warm). po2's first
                # unit runs ahead of the normalizes: its matmuls cover the
                # PE-idle window while DVE finishes pair-1's reciprocal,
                # without delaying the A2A(3) trigger.
                po_unit(2, yg2, 0, 0)()
                norm_h(3, rec[3], 2)()
                norm_h(3, rec[3], 3)()
                stage_a2a(3)()
                for i, (b, ns) in enumerate(((0, 1), (1, 0), (1, 1))):
                    po_unit(2, yg2, b, ns, alt=(i % 2 == 0))()
                for f in opf[8:]:
                    f()
                dps = sc_psum.tile([128, SP], f32, tag="sc", name="dummy")
                for _ in range(20):
                    nc.tensor.matmul(dps, lhsT=warm_sb[:, 0:128],
                                     rhs=warm_sb, start=True, stop=True)
                yg3 = gather_span(3, "yg3")
                for i, (b, ns) in enumerate(((0, 0), (0, 1), (1, 0), (1, 1))):
                    po_unit(3, yg3, b, ns, alt=(i % 2 == 1))()

    nc.compile()
    return nc


def _get_program():
    if "nc" not in _CACHE:
        _CACHE["nc"] = _build_program()
    return _CACHE["nc"]


def _make_in_maps(x, mask, Wq, bq, Wk, bk, Wv, bv, Wo, bo):
    x = np.asarray(x, np.float32)
    mask = np.asarray(mask, bool)
    Wq = np.asarray(Wq, np.float32)
    Wk = np.asarray(Wk, np.float32)
    Wv = np.asarray(Wv, np.float32)
    Wo = np.asarray(Wo, np.float32)
    bq = np.asarray(bq, np.float32)
    bk = np.asarray(bk, np.float32)
    bv = np.asarray(bv, np.float32)
    bo = np.asarray(bo, np.float32)

    woP = np.ascontiguousarray(
        Wo.T.reshape(8, 128, D).transpose(1, 0, 2)).astype(BF16)
    in_maps = []
    per_batch = {}
    for b in range(B):
        xTb = np.ascontiguousarray(x[b].T)
        # the only masking the kernel applies is the 128x128 diagonal
        # block (identical for every diagonal tile of a causal mask)
        md = mask[b, 0].T[0:128, 0:128].astype(np.float32)
        per_batch[b] = (xTb, md)
    for c in range(NCORES):
        b, g = divmod(c, GROUPS)
        sl = slice(g * DL, (g + 1) * DL)
        xTb, md = per_batch[b]
        in_maps.append({
            "xP": np.ascontiguousarray(
                xTb.reshape(8, 128, 2, T // 2).transpose(1, 2, 0, 3)
            ).astype(BF16),
            "wqP": np.ascontiguousarray(
                (Wq[sl] * SCALE).T.reshape(8, 128, DL).transpose(1, 0, 2)
            ).astype(BF16),
            "wkP": np.ascontiguousarray(
                Wk[sl].T.reshape(8, 128, DL).transpose(1, 0, 2)).astype(BF16),
            "wvP": np.ascontiguousarray(
                Wv[sl].T.reshape(8, 128, DL).transpose(1, 0, 2)).astype(BF16),
            "woP": woP,
            "bqP": np.ascontiguousarray((bq[sl] * SCALE).reshape(2, 128).T),
            "bkP": np.ascontiguousarray(bk[sl].reshape(2, 128).T),
            "bv": bv[sl].reshape(1, DL).astype(BF16),
            "bo": bo.reshape(1, D).astype(np.float32),
            "maskd": md.astype(BF16),
            "onesb": np.ones((1, 128), BF16),
        })
    return in_maps


def _capture_profile(nc, in_maps, tmpdir):
    """Run with NTFF capture and process the profile ourselves. Returns
    (results, exec_time_ns|None)."""
    import glob
    import json
    import re
    import subprocess
    from trn_agent_boot.trn_boot import _ntff_profile_via_ctypes
    from concourse import bass2jax

    hook = _ntff_profile_via_ctypes("/opt/axon/libaxon_pjrt.so")
    if hook is None:
        raise RuntimeError("libaxon_pjrt.so lacks NTFF profile symbols")
    os.makedirs(tmpdir, exist_ok=True)
    with hook(tmpdir, [0]):
        results = bass2jax.run_bass_via_pjrt(nc, in_maps, n_cores=NCORES)

    ntffs = glob.glob(os.path.join(tmpdir, "*_body*-device*.ntff"))
    best = None
    for f in ntffs:
        if re.search(r"executable(\d+)-device000000", f):
            if best is None or os.path.getmtime(f) > os.path.getmtime(best):
                best = f
    if best is None:
        raise RuntimeError(f"no NTFF produced in {tmpdir}")
    neff = re.sub(r"-device\d+-execution-\d+\.ntff$", ".neff", best)
    out_json = os.path.join(tmpdir, "prof.json")
    subprocess.check_call(
        ["neuron-profile", "view", "--ignore-nc-buf-usage", "-s", best,
         "-n", neff, "--output-format=json", f"--output-file={out_json}"],
        cwd=tmpdir)
    summary = json.load(open(out_json))["summary"][0]
    return results, int(summary["total_time"] * 1e9)


def kernel(x, mask, Wq, bq, Wk, bk, Wv, bv, Wo, bo):
    from concourse import bass_utils

    in_maps = _make_in_maps(x, mask, Wq, bq, Wk, bk, Wv, bv, Wo, bo)
    nc = _get_program()

    trace = bool(int(os.environ.get("MHA_TRACE", "0")))
    tmpdir = os.environ.get("MHA_TRACE_DIR") or None
    results = None
    if trace and tmpdir:
        try:
            results, exec_ns = _capture_profile(nc, in_maps, tmpdir)
            _CACHE["last_exec_time_ns"] = exec_ns
        except Exception as e:  # profiling is best-effort
            print(f"profiling unavailable: {type(e).__name__}: {e}")
            results = None
    if results is None:
        results = bass_utils.run_bass_kernel_spmd(
            nc, in_maps, core_ids=list(range(NCORES))).results
        _CACHE.setdefault("last_exec_time_ns", None)

    # core c's out[qs] holds rows (q = qs*512 + c*64 + i) for batch 0
    # (rows 0-63) and batch 1 (rows 64-127)
    out = np.empty((B, T, D), np.float32)
    for c in range(NCORES):
        o = np.asarray(results[c]["out"], np.float32)
        for qs in range(QS):
            q0 = qs * SP + c * QB
            out[0, q0:q0 + QB] = o[qs, 0:QB]
            out[1, q0:q0 + QB] = o[qs, QB:2 * QB]
    return out


# revision 38
# speedup vs baseline: 1.0711x; 1.0711x over previous
"""Causal multi-head attention (B=2, T=2048, D=1024, H=16) on 8 TRN2 NeuronCores.

Sharding: core c = (batch b = c//4, head-group g = c%4). Each core owns 4 heads
(= 256 contiguous dims of D) of one batch: Megatron-style tensor parallelism on
heads x data parallelism on batch.

Design (single fully-pipelined phase; everything but span 0's attention is
emitted as "fillers" interleaved into the attention kt loops so the in-order
PE queue has no phase boundaries and never head-of-line blocks):
  - Out-projection reduction via per-q-span 8-way bf16 AllToAll of the
    normalized attention output yT (rank r's territory = 64-col q-block r of
    each 512-q span, for BOTH batches -- SPMD-uniform, no junk shards). Each
    core then out-projects its territory with the full Wo.
  - Score matmuls pack the two heads of an mc-chunk as two concurrent K=64
    row-group tiles into one [128, 2*512] 2-bank PSUM tile; ONE exp per pair.
    The kt loop software-pipelines scores kt+1 ahead of the AV matmuls of kt;
    fillers pop BEFORE the exp-gated AV matmuls so the in-order PE queue keeps
    running independent work while ACT produces exp(kt). ACT runs ONLY exp --
    a single table set, loaded once at warmup, zero mid-run table switches.
  - AV uses the v_aug 65th-column trick (denominator accumulates as row 64);
    1/den via the single-instruction DVE approx reciprocal (~18 bits, plenty
    upstream of bf16); normalization = PE rank-1 broadcast of 1/den +
    in-place DVE multiply, one span behind attention.
  - Inputs use pre-chunked partition-major DRAM layouts (host-side prep) so
    each tensor loads as a handful of DMAs with 2-16KB per-partition packets
    (the DMA engines are packet-rate-bound below ~2KB): wq + x-first-half
    land ~4us after the queues arm and span-0 projections start immediately;
    a short PE warmup stream covers the arm window (HAM un-throttle). After
    the lead, bulk loads ride sync/gpsimd only -- NEVER the scalar queue,
    whose instruction stream must stay free for the exp activations.
  - Span 0/1/2 normalizes + A2A triggers fire directly at span end (the
    collective stream can back up by tens of us in bad runs; every exchange
    gets a full span of slack before its out-projection consumers).
  - Out-projection: pair 0 = spans 0+1 stacked (M=128), b=0 units spread
    over span 3's kt loop as fillers, b=1 units deferred into the A2A(3)
    flight. Spans 2 and 3 use per-(b,ns) M=64 units: span 2's depend only on
    A2A(2) and fill the A2A(3) flight (with a short dummy-matmul bridge to
    keep HAM warm); span 3's follow the single full-span A2A(3) that fires
    right after the last normalize. bo folds in on DVE at PSUM evacuation
    (or via a rank-1 seed + ACT copy on alternating tail units so the final
    evacuations run on two engines). Output DMAs alternate gpsimd/sync so
    they never queue behind gather DMAs.

Dtypes: all matmul operands bf16 with fp32 PSUM accumulation; softmax exp(s)
without row-max (scores O(1), scale folded into Wq host-side).
"""

import os
import numpy as np
import ml_dtypes

BF16 = ml_dtypes.bfloat16

B, T, D, H = 2, 2048, 1024, 16
HD = D // H                     # 64
NCORES = 8
GROUPS = 4                      # cores per batch (tensor-parallel degree)
HL = H // GROUPS                # heads per core = 4
DL = D // GROUPS                # dims per core = 256
SP = 512                        # free-dim span per matmul (one PSUM bank, fp32)
QS = T // SP                    # 4 q spans
KT = T // 128                   # 16 k tiles
QB = 64                         # q columns per rank territory per span
SCALE = HD ** -0.5

_CACHE = {}


def _build_program():
    import concourse.bass as bass  # noqa: F401  (registers bass machinery)
    import concourse.tile as tile
    from concourse import bacc, mybir

    f32 = mybir.dt.float32
    bf16 = mybir.dt.bfloat16
    Exp = mybir.ActivationFunctionType.Exp

    nc = bacc.Bacc("TRN2", target_bir_lowering=False, debug=False,
                   num_devices=NCORES)

    # pre-chunked host-side layouts: partition-major so each tensor loads
    # as a handful of DMAs whose per-partition runs are 2-16KB (the DMA
    # engines are packet-rate-bound below ~2KB)
    xP = nc.dram_tensor("xP", [128, 2, 8, 2 * SP], bf16, kind="ExternalInput")
    wqP = nc.dram_tensor("wqP", [128, 8, DL], bf16, kind="ExternalInput")
    wkP = nc.dram_tensor("wkP", [128, 8, DL], bf16, kind="ExternalInput")
    wvP = nc.dram_tensor("wvP", [128, 8, DL], bf16, kind="ExternalInput")
    woP = nc.dram_tensor("woP", [128, 8, D], bf16, kind="ExternalInput")
    bqP = nc.dram_tensor("bqP", [128, 2], f32, kind="ExternalInput")
    bkP = nc.dram_tensor("bkP", [128, 2], f32, kind="ExternalInput")
    bv = nc.dram_tensor("bv", [1, DL], bf16, kind="ExternalInput")
    bo = nc.dram_tensor("bo", [1, D], f32, kind="ExternalInput")
    maskd = nc.dram_tensor("maskd", [128, 128], bf16, kind="ExternalInput")
    onesb = nc.dram_tensor("onesb", [1, 128], bf16, kind="ExternalInput")
    out_ext = nc.dram_tensor("out", [QS, 128, D], bf16,
                             kind="ExternalOutput")

    ALL8 = [[0, 1, 2, 3, 4, 5, 6, 7]]

    with tile.TileContext(nc) as tc:
        with tc.tile_pool(name="main", bufs=1) as main, \
             tc.tile_pool(name="dram", bufs=1, space="DRAM") as dram:
            qT_s = main.tile([128, 2, T], bf16)
            kT_s = main.tile([128, 2, T], bf16)
            v_s = main.tile([128, KT, HL * 65], bf16)
            yT_s = main.tile([128, 2, T], bf16)
            woT_s = main.tile([128, 8, D], bf16)
            bq_s = main.tile([128, 2], f32)
            bk_s = main.tile([128, 2], f32)
            bo_bc = main.tile([128, D], f32)
            onesb_s = main.tile([128, 128], bf16)
            bv_bc = main.tile([128, DL], bf16)
            maskd_s = main.tile([128, 128], bf16)
            warm_s = main.tile([128, 2], f32)
            bo_row = main.tile([1, D], bf16)
            warm_sb = main.tile([128, SP], bf16)

            # per-span A2A staging (separate tiles avoid false DRAM deps)
            a2a_in = [dram.tile([8, DL, QB], bf16, name=f"a2ain{i}")
                      for i in range(QS)]
            a2a_out = [dram.tile([8, DL, QB], bf16, name=f"a2aout{i}")
                       for i in range(QS)]

            # PE warmup: back-to-back matmuls on scratch data while the first
            # input DMAs stream in (HAM un-throttle); real projection matmuls
            # take over as soon as their kc operands land
            nc.vector.memset(warm_sb, 1.0)
            with tc.tile_pool(name="warm_psum", bufs=1,
                              space="PSUM") as warm_psum:
                wps = warm_psum.tile([128, SP], f32, tag="w")
                for _ in range(34):
                    nc.tensor.matmul(wps, lhsT=warm_sb[:, 0:128],
                                     rhs=warm_sb, start=True, stop=True)

            # tiny high-priority loads on the sync queue
            nc.sync.dma_start(out=bq_s, in_=bqP[:])
            nc.sync.dma_start(out=bk_s, in_=bkP[:])
            # pre-load the ACT Exp table (the only set the kernel ever uses)
            nc.scalar.activation(warm_s, warm_sb[:, 0:2], Exp)
            # small loads on the scalar queue (bo/onesb wait until after
            # the lead x/wq loads -- they are not needed until much later)
            nc.scalar.dma_start(out=bv_bc, in_=bv[:].to_broadcast([128, DL]))
            # ones column at index 64 of each head's 65-wide block of v_aug
            nc.vector.memset(v_s, 1.0)

            # ---------------- input loads ----------------
            xt_h = main.tile([128, 2, 8, 2 * SP], bf16)
            wq_s = main.tile([128, 8, DL], bf16)
            wk_s = main.tile([128, 8, DL], bf16)
            wv_s = main.tile([128, 8, DL], bf16)

            # lead: wq + x first half, balanced by measured queue speed
            # (gpsimd ~2x faster than sync, scalar in between)
            nc.sync.dma_start(out=wq_s[:], in_=wqP[:])
            nc.gpsimd.dma_start(out=xt_h[:, 0, 0:4, :], in_=xP[:, 0, 0:4, :])
            nc.scalar.dma_start(out=xt_h[:, 0, 4:7, :], in_=xP[:, 0, 4:7, :])
            nc.sync.dma_start(out=xt_h[:, 0, 7:8, :], in_=xP[:, 0, 7:8, :])
            # mid: wk/wv + small tiles (scalar queue stays free for the exp
            # stream from here on)
            nc.gpsimd.dma_start(out=wk_s[:], in_=wkP[:])
            nc.scalar.dma_start(out=wv_s[:], in_=wvP[:])
            nc.sync.dma_start(out=maskd_s, in_=maskd[:])
            nc.sync.dma_start(out=onesb_s,
                              in_=onesb[:].to_broadcast([128, 128]))
            nc.sync.dma_start(out=bo_bc, in_=bo[:].to_broadcast([128, D]))
            # tail loads: x second half + woT on sync/gpsimd
            nc.gpsimd.dma_start(out=xt_h[:, 1, 0:5, :], in_=xP[:, 1, 0:5, :])
            nc.sync.dma_start(out=xt_h[:, 1, 5:8, :], in_=xP[:, 1, 5:8, :])
            nc.gpsimd.dma_start(out=woT_s[:, 0:5, :], in_=woP[:, 0:5, :])
            nc.sync.dma_start(out=woT_s[:, 5:8, :], in_=woP[:, 5:8, :])

            # bf16 copy of bo for the rank-1 out-projection seeds
            nc.vector.tensor_copy(bo_row, bo_bc[0:1, :])


            # ---- single pipelined phase ----
            with tc.tile_pool(name="attn_t", bufs=4) as attn_t, \
                 tc.tile_pool(name="nrm", bufs=2) as nrm, \
                 tc.tile_pool(name="op_sb", bufs=4) as op_sb, \
                 tc.tile_pool(name="sc_psum", bufs=2, space="PSUM") as sc_psum, \
                 tc.tile_pool(name="av_psum", bufs=2, space="PSUM") as av_psum, \
                 tc.tile_pool(name="op_psum", bufs=2, space="PSUM") as op_psum:

                def attention_span(qs, fillers, self_norm=False):
                    # denominator rows at partitions 0/32/64/96 (engine APs
                    # must start 32-aligned); memset keeps unused rows finite
                    den_stack = nrm.tile([97, SP], f32, tag="den")
                    nc.vector.memset(den_stack, 1.0)
                    rec32 = nrm.tile([97, SP], f32, tag="rec32")
                    rec_bf = nrm.tile([97, SP], bf16, tag="recf")
                    nkt = 4 * qs + 4  # causal: later k tiles are all-masked
                    span = slice(qs * SP, (qs + 1) * SP)
                    # carry pre-load: pop ~2 fillers in the first steps of
                    # the span, where the PE would otherwise micro-idle
                    # waiting on exp(0) and HAM would re-throttle
                    pace = {"left": 2 * nkt, "carry": 2.0}

                    def pop_fillers():
                        frac = pace["carry"] + len(fillers) / pace["left"]
                        n_pop = int(frac)
                        pace["carry"] = frac - n_pop
                        pace["left"] -= 1
                        for _ in range(min(n_pop, len(fillers))):
                            fillers.pop(0)()

                    for p in range(2):  # head pair = mc chunk p
                        qa = qT_s[0:64, p, span]
                        qb = qT_s[64:128, p, span]
                        ya = av_psum.tile([65, SP], f32, tag="av")
                        yb = av_psum.tile([65, SP], f32, tag="av")

                        def sc_pair(kt):
                            # diagonal tiles: q columns < 128*(kt-4qs) are
                            # fully masked; stream only the valid sub-range
                            j0 = max(0, (kt - 4 * qs) * 128)
                            scp = sc_psum.tile([128, 2 * SP], f32, tag="sc")
                            nc.tensor.matmul(
                                scp[:, j0:SP],
                                lhsT=kT_s[0:64, p, kt * 128:(kt + 1) * 128],
                                rhs=qa[:, j0:SP], start=True, stop=True)
                            nc.tensor.matmul(
                                scp[:, SP + j0:2 * SP],
                                lhsT=kT_s[64:128, p, kt * 128:(kt + 1) * 128],
                                rhs=qb[:, j0:SP], start=True, stop=True)
                            return scp

                        # software pipeline: scores kt+1 are emitted before
                        # the AV matmuls of kt so the in-order PE queue keeps
                        # feeding ACT while AV waits on exp kt
                        scp = sc_pair(0)
                        for kt in range(nkt):
                            atp = attn_t.tile([128, 2 * SP], bf16, tag="at")
                            j0e = max(0, (kt - 4 * qs) * 128)
                            if j0e >= 256:
                                # mostly-masked diagonal tile: two narrow
                                # exps over the valid ranges beat one full-
                                # width one
                                nc.scalar.activation(atp[:, j0e:SP],
                                                     scp[:, j0e:SP], Exp)
                                nc.scalar.activation(
                                    atp[:, SP + j0e:2 * SP],
                                    scp[:, SP + j0e:2 * SP], Exp)
                            else:
                                nc.scalar.activation(atp, scp, Exp)
                            if kt >= 4 * qs:
                                # diagonal tile: only its 128-col diagonal
                                # block needs masking and that block is the
                                # same tril(128) for every tile
                                jm = (kt - 4 * qs) * 128
                                nc.vector.tensor_mul(
                                    atp[:, jm:jm + 128],
                                    atp[:, jm:jm + 128], maskd_s)
                                nc.vector.tensor_mul(
                                    atp[:, SP + jm:SP + jm + 128],
                                    atp[:, SP + jm:SP + jm + 128], maskd_s)
                            if kt + 1 < nkt:
                                scp = sc_pair(kt + 1)
                            # independent fillers run while ACT produces
                            # exp(kt); they must precede the exp-gated AVs
                            pop_fillers()
                            j0 = max(0, (kt - 4 * qs) * 128)
                            nc.tensor.matmul(
                                ya[:, j0:SP],
                                lhsT=v_s[:, kt, (2 * p) * 65:
                                             (2 * p + 1) * 65],
                                rhs=atp[:, j0:SP],
                                start=(kt == 0), stop=(kt == nkt - 1))
                            nc.tensor.matmul(
                                yb[:, j0:SP],
                                lhsT=v_s[:, kt, (2 * p + 1) * 65:
                                             (2 * p + 2) * 65],
                                rhs=atp[:, SP + j0:2 * SP],
                                start=(kt == 0), stop=(kt == nkt - 1))
                        # evacuate unnormalized yT' + denominators so the
                        # PSUM banks free for the next pair; the last pair's
                        # evacuation is on the A2A(3)-trigger critical path,
                        # so split it across DVE and the idle ACT engine
                        if self_norm and p == 1:
                            # denominators first on DVE (they gate the
                            # reciprocal -> normalize -> A2A(3) trigger);
                            # the y copies ride the idle ACT engine
                            nc.vector.tensor_copy(
                                den_stack[64 * p:64 * p + 1, :], ya[64:65, :])
                            nc.vector.tensor_copy(
                                den_stack[64 * p + 32:64 * p + 33, :],
                                yb[64:65, :])
                            nc.scalar.copy(yT_s[0:64, p, span], ya[0:64, :])
                            nc.scalar.copy(yT_s[64:128, p, span],
                                           yb[0:64, :])
                        else:
                            # ya's copies first: the next pair's first AV
                            # reuses ya's PSUM slot and shouldn't wait for
                            # yb's evacuation too
                            nc.vector.tensor_copy(yT_s[0:64, p, span],
                                                  ya[0:64, :])
                            nc.vector.tensor_copy(
                                den_stack[64 * p:64 * p + 1, :], ya[64:65, :])
                            nc.vector.tensor_copy(yT_s[64:128, p, span],
                                                  yb[0:64, :])
                            nc.vector.tensor_copy(
                                den_stack[64 * p + 32:64 * p + 33, :],
                                yb[64:65, :])
                        if self_norm:
                            # last span: full-width DVE reciprocal per pair
                            # (the custom DVE op wants partition offset 0;
                            # pair-1 rows are memset 1.0 until valid and the
                            # second pass overwrites everything)
                            nc.vector.reciprocal_approx_fast(
                                out=rec32, in_=den_stack)
                            nc.vector.tensor_copy(rec_bf, rec32)
                            if p == 0:
                                # front-insert: the mc0 normalize must fire
                                # early in pair 1's sweep
                                fillers[0:0] = [norm_h(qs, rec_bf, 0),
                                                norm_h(qs, rec_bf, 1)]
                                pace["carry"] += 2.0
                    if not self_norm:
                        # single-instruction DVE reciprocal: no ACT table
                        # switches, no span-boundary exp contention
                        nc.vector.reciprocal_approx_fast(out=rec32,
                                                         in_=den_stack)
                        nc.vector.tensor_copy(rec_bf, rec32)
                    return rec_bf

                def norm_h(qs, rec_bf, h):
                    def f():
                        span = slice(qs * SP, (qs + 1) * SP)
                        mc, r0 = divmod(h, 2)
                        r0 *= 64
                        rb = op_psum.tile([64, SP], f32, tag="op")
                        r0p = 32 * h
                        nc.tensor.matmul(rb,
                                         lhsT=onesb_s[r0p:r0p + 1, 0:64],
                                         rhs=rec_bf[r0p:r0p + 1, :],
                                         start=True, stop=True,
                                         tile_position=(r0p, 0))
                        nc.vector.tensor_mul(yT_s[r0:r0 + 64, mc, span],
                                             yT_s[r0:r0 + 64, mc, span],
                                             rb)
                    return f

                def stage_a2a(qs):
                    def f():
                        span = slice(qs * SP, (qs + 1) * SP)
                        in_r = a2a_in[qs][:].rearrange(
                            "j (two p) q -> two p j q", p=128)
                        # the staging fragments into 128B packets (per-rank
                        # chunks split every partition row): run the two mc
                        # halves on different queues so the trigger waits
                        # half as long
                        for mc, q in ((0, nc.sync), (1, nc.gpsimd)):
                            q.dma_start(
                                out=in_r[mc],
                                in_=yT_s[:, mc, span].rearrange(
                                    "p (j q) -> p j q", q=QB))
                        nc.gpsimd.collective_compute(
                            "AllToAll", mybir.AluOpType.bypass,
                            replica_groups=ALL8,
                            ins=[a2a_in[qs][:].opt()],
                            outs=[a2a_out[qs][:].opt()])
                    return f

                def proj_qk(w_s, b_s, dst, mc, s):
                    # qT/kT group: out[dims-chunk mc, t-span s]; bias added
                    # on DVE during the PSUM->SBUF evacuation
                    def f():
                        ps = op_psum.tile([128, SP], f32, tag="op",
                                          name="pj")
                        t0 = (s % 2) * SP
                        for kc in range(8):
                            nc.tensor.matmul(
                                ps,
                                lhsT=w_s[:, kc, mc * 128:(mc + 1) * 128],
                                rhs=xt_h[:, s // 2, kc, t0:t0 + SP],
                                start=(kc == 0), stop=(kc == 7))
                        nc.vector.tensor_scalar_add(
                            dst[:, mc, s * SP:(s + 1) * SP],
                            ps, b_s[:, mc:mc + 1])
                    return f

                def proj_v(mt):
                    # v tile in natural [t, d] layout; bias via DVE add into
                    # the 65-stride v_aug slots
                    def f():
                        ps = op_psum.tile([128, SP], f32, tag="op",
                                          name="pjv")
                        t0 = (mt % 8) * 128
                        for kc in range(8):
                            nc.tensor.matmul(
                                ps[:, 0:DL],
                                lhsT=xt_h[:, mt // 8, kc, t0:t0 + 128],
                                rhs=wv_s[:, kc, :],
                                start=(kc == 0), stop=(kc == 7))
                        nc.vector.tensor_add(
                            v_s[:, mt, :].rearrange(
                                "p (h d) -> p h d", d=65)[:, :, 0:64],
                            ps[:, 0:DL].rearrange("p (h d) -> p h d", d=64),
                            bv_bc.rearrange("p (h d) -> p h d", d=64))
                    return f

                def proj_span_fillers(s):
                    fs = []
                    for mc in range(2):
                        fs.append(proj_qk(wq_s, bq_s, qT_s, mc, s))
                    for mc in range(2):
                        fs.append(proj_qk(wk_s, bk_s, kT_s, mc, s))
                    for mt in range(4 * s, 4 * s + 4):
                        fs.append(proj_v(mt))
                    return fs

                def outproj_pair0_fillers():
                    """Out-projection for spans (0,1) with M=128 (both spans'
                    64-q territories stacked), split into per-(b,ns) filler
                    chunks; spread over span 3's kt loop."""
                    yg = op_sb.tile([128, 8, 2, 2 * QB], bf16, tag="yg",
                                    bufs=1)

                    def gather(sp):
                        def f():
                            out_r = a2a_out[sp][:].rearrange(
                                "(b j2) (h p) q -> b p (j2 h) q", j2=4, p=128)
                            m = sp % 2
                            # 128B-packet gathers: split batches over two
                            # queues so the po consumers wait half as long
                            for b, q in ((0, nc.sync), (1, nc.gpsimd)):
                                q.dma_start(
                                    out=yg[:, :, b, m * QB:(m + 1) * QB],
                                    in_=out_r[b])
                        return f

                    def po_mms(b, ns, kcs, box, last=False):
                        def f():
                            if not box:
                                box.append(op_psum.tile(
                                    [128, SP], f32, tag="op", name="po"))
                            po = box[0]
                            for kc in kcs:
                                nc.tensor.matmul(
                                    po, lhsT=yg[:, kc, b, :],
                                    rhs=woT_s[:, kc, ns * SP:(ns + 1) * SP],
                                    start=(kc == 0),
                                    stop=(last and kc == kcs[-1]))
                            if last:
                                ob = op_sb.tile([128, SP], bf16, tag="ob")
                                nc.vector.tensor_add(
                                    ob, po, bo_bc[:, ns * SP:(ns + 1) * SP])
                                for m in range(2):
                                    nc.gpsimd.dma_start(
                                        out=out_ext[m, b * 64:(b + 1) * 64,
                                                    ns * SP:(ns + 1) * SP],
                                        in_=ob[m * 64:(m + 1) * 64, :])
                        return f

                    fs = [gather(0), gather(1)]
                    for b in range(2):
                        for ns in range(2):
                            box = []
                            fs.append(po_mms(b, ns, [0, 1, 2], box))
                            fs.append(po_mms(b, ns, [3, 4, 5], box))
                            fs.append(po_mms(b, ns, [6, 7], box, last=True))
                    return fs

                def gather_span(sp, tag):
                    yg = op_sb.tile([128, 8, 2, QB], bf16, tag=tag, bufs=1,
                                    name=tag)
                    out_r = a2a_out[sp][:].rearrange(
                        "(b j2) (h p) q -> b p (j2 h) q", j2=4, p=128)
                    for b, q in ((0, nc.sync), (1, nc.gpsimd)):
                        q.dma_start(out=yg[:, :, b, :], in_=out_r[b])
                    return yg

                def po_unit(sp, yg, b, ns, alt=False):
                    # M=64 out-projection of span sp's territory; engines
                    # alternate so the teardown isn't single-queue-bound.
                    def f():
                        po = op_psum.tile([64, SP], f32, tag="op", name="pou")
                        if alt:
                            # fold bo via a rank-1 seed so the evacuation is
                            # a plain copy the otherwise-idle ACT engine can
                            # run in parallel with DVE's add-evacuations
                            nc.tensor.matmul(
                                po, lhsT=onesb_s[0:1, 0:64],
                                rhs=bo_row[0:1, ns * SP:(ns + 1) * SP],
                                start=True, stop=False)
                        for kc in range(8):
                            nc.tensor.matmul(
                                po, lhsT=yg[:, kc, b, :],
                                rhs=woT_s[:, kc, ns * SP:(ns + 1) * SP],
                                start=(not alt and kc == 0),
                                stop=(kc == 7))
                        ob = op_sb.tile([64, SP], bf16, tag="ob")
                        if alt:
                            nc.scalar.copy(ob, po)
                        else:
                            nc.vector.tensor_add(
                                ob, po, bo_bc[0:64, ns * SP:(ns + 1) * SP])
                        dq = nc.sync if alt else nc.gpsimd
                        dq.dma_start(
                            out=out_ext[sp, b * 64:(b + 1) * 64,
                                        ns * SP:(ns + 1) * SP],
                            in_=ob)
                    return f

                # prologue: only q-mc0/k-mc0 (all the first score matmul
                # needs) run inline; every v unit rides the kt loop as an
                # early filler so the in-order PE queue never stalls on the
                # later-arriving wv load (the pacing front-bias guarantees
                # v(kt) pops before AV(kt))
                p0 = proj_span_fillers(0)
                for f in [p0[0], p0[2]]:                # q0, k0
                    f()
                pend = [p0[4], p0[5], p0[6], p0[7],     # v0..v3
                        p0[1], p0[3]]                   # q1, k1
                rec = {}
                for qs in range(QS):
                    if qs < 3:
                        # span qs+1's projections drain during span qs
                        pend.extend(proj_span_fillers(qs + 1))
                    else:
                        # A2A(0)/(1) completed spans ago: pair-0's b=0
                        # out-projection spreads over span 3's kt loop; the
                        # b=1 units are deferred into the A2A(3) flight
                        opf = outproj_pair0_fillers()
                        pend.extend(opf[:8])

                        def dummy_filler():
                            def f():
                                dp = op_psum.tile([128, SP], f32, tag="op",
                                                  name="dumf")
                                nc.tensor.matmul(dp, lhsT=warm_sb[:, 0:128],
                                                 rhs=warm_sb, start=True,
                                                 stop=True)
                            return f
                        # span 3's late kt steps are ACT-bound with the
                        # filler list nearly drained: pad with dummy matmuls
                        # so the PE never micro-idles (HAM stays warm)
                        pend.extend(dummy_filler() for _ in range(6))
                    if qs == 3:
                        yg2 = gather_span(2, "yg2")
                    rec[qs] = attention_span(qs, pend, self_norm=(qs == 3))
                    if qs < 3:
                        # normalize + trigger the exchange right at span end:
                        # the collective stream can back up by tens of us in
                        # bad runs, and every A2A must land well before its
                        # out-projection consumers
                        for h in range(HL):
                            norm_h(qs, rec[qs], h)()
                        stage_a2a(qs)()
                for f in pend:
                    f()

                # ---- tail: normalize span-3 mc1, fire the single full-span
                # A2A(3), then fill its ~20us flight with span-2's
                # out-projection, pair-0's deferred b=1 units, and a short
                # dummy-matmul bridge (keeps HAM at full clock so the
                # A2A-gated span-3 out-projection runs warm). po2's first
                # unit runs ahead of the normalizes: its matmuls cover the
                # PE-idle window while DVE finishes pair-1's reciprocal,
                # without delaying the A2A(3) trigger.
                po_unit(2, yg2, 0, 0)()
                norm_h(3, rec[3], 2)()
                norm_h(3, rec[3], 3)()
                stage_a2a(3)()
                for i, (b, ns) in enumerate(((0, 1), (1, 0), (1, 1))):
                    po_unit(2, yg2, b, ns, alt=(i % 2 == 0))()
                for f in opf[8:]:
                    f()
                dps = sc_psum.tile([128, SP], f32, tag="sc", name="dummy")
                for _ in range(20):
                    nc.tensor.matmul(dps, lhsT=warm_sb[:, 0:128],
                                     rhs=warm_sb, start=True, stop=True)
                yg3 = gather_span(3, "yg3")
                for i, (b, ns) in enumerate(((0, 0), (0, 1), (1, 0), (1, 1))):
                    po_unit(3, yg3, b, ns, alt=(i % 2 == 1))()

    nc.compile()
    return nc


def _get_program():
    if "nc" not in _CACHE:
        _CACHE["nc"] = _build_program()
    return _CACHE["nc"]


def _make_in_maps(x, mask, Wq, bq, Wk, bk, Wv, bv, Wo, bo):
    x = np.asarray(x, np.float32)
    mask = np.asarray(mask, bool)
    Wq = np.asarray(Wq, np.float32)
    Wk = np.asarray(Wk, np.float32)
    Wv = np.asarray(Wv, np.float32)
    Wo = np.asarray(Wo, np.float32)
    bq = np.asarray(bq, np.float32)
    bk = np.asarray(bk, np.float32)
    bv = np.asarray(bv, np.float32)
    bo = np.asarray(bo, np.float32)

    woP = np.ascontiguousarray(
        Wo.T.reshape(8, 128, D).transpose(1, 0, 2)).astype(BF16)
    in_maps = []
    per_batch = {}
    for b in range(B):
        xTb = np.ascontiguousarray(x[b].T)
        # the only masking the kernel applies is the 128x128 diagonal
        # block (identical for every diagonal tile of a causal mask)
        md = mask[b, 0].T[0:128, 0:128].astype(np.float32)
        per_batch[b] = (xTb, md)
    for c in range(NCORES):
        b, g = divmod(c, GROUPS)
        sl = slice(g * DL, (g + 1) * DL)
        xTb, md = per_batch[b]
        in_maps.append({
            "xP": np.ascontiguousarray(
                xTb.reshape(8, 128, 2, T // 2).transpose(1, 2, 0, 3)
            ).astype(BF16),
            "wqP": np.ascontiguousarray(
                (Wq[sl] * SCALE).T.reshape(8, 128, DL).transpose(1, 0, 2)
            ).astype(BF16),
            "wkP": np.ascontiguousarray(
                Wk[sl].T.reshape(8, 128, DL).transpose(1, 0, 2)).astype(BF16),
            "wvP": np.ascontiguousarray(
                Wv[sl].T.reshape(8, 128, DL).transpose(1, 0, 2)).astype(BF16),
            "woP": woP,
            "bqP": np.ascontiguousarray((bq[sl] * SCALE).reshape(2, 128).T),
            "bkP": np.ascontiguousarray(bk[sl].reshape(2, 128).T),
            "bv": bv[sl].reshape(1, DL).astype(BF16),
            "bo": bo.reshape(1, D).astype(np.float32),
            "maskd": md.astype(BF16),
            "onesb": np.ones((1, 128), BF16),
        })
    return in_maps


def _capture_profile(nc, in_maps, tmpdir):
    """Run with NTFF capture and process the profile ourselves. Returns
    (results, exec_time_ns|None)."""
    import glob
    import json
    import re
    import subprocess
    from trn_agent_boot.trn_boot import _ntff_profile_via_ctypes
    from concourse import bass2jax

    hook = _ntff_profile_via_ctypes("/opt/axon/libaxon_pjrt.so")
    if hook is None:
        raise RuntimeError("libaxon_pjrt.so lacks NTFF profile symbols")
    os.makedirs(tmpdir, exist_ok=True)
    with hook(tmpdir, [0]):
        results = bass2jax.run_bass_via_pjrt(nc, in_maps, n_cores=NCORES)

    ntffs = glob.glob(os.path.join(tmpdir, "*_body*-device*.ntff"))
    best = None
    for f in ntffs:
        if re.search(r"executable(\d+)-device000000", f):
            if best is None or os.path.getmtime(f) > os.path.getmtime(best):
                best = f
    if best is None:
        raise RuntimeError(f"no NTFF produced in {tmpdir}")
    neff = re.sub(r"-device\d+-execution-\d+\.ntff$", ".neff", best)
    out_json = os.path.join(tmpdir, "prof.json")
    subprocess.check_call(
        ["neuron-profile", "view", "--ignore-nc-buf-usage", "-s", best,
         "-n", neff, "--output-format=json", f"--output-file={out_json}"],
        cwd=tmpdir)
    summary = json.load(open(out_json))["summary"][0]
    return results, int(summary["total_time"] * 1e9)


def kernel(x, mask, Wq, bq, Wk, bk, Wv, bv, Wo, bo):
    from concourse import bass_utils

    in_maps = _make_in_maps(x, mask, Wq, bq, Wk, bk, Wv, bv, Wo, bo)
    nc = _get_program()

    trace = bool(int(os.environ.get("MHA_TRACE", "0")))
    tmpdir = os.environ.get("MHA_TRACE_DIR") or None
    results = None
    if trace and tmpdir:
        try:
            results, exec_ns = _capture_profile(nc, in_maps, tmpdir)
            _CACHE["last_exec_time_ns"] = exec_ns
        except Exception as e:  # profiling is best-effort
            print(f"profiling unavailable: {type(e).__name__}: {e}")
            results = None
    if results is None:
        results = bass_utils.run_bass_kernel_spmd(
            nc, in_maps, core_ids=list(range(NCORES))).results
        _CACHE.setdefault("last_exec_time_ns", None)

    # core c's out[qs] holds rows (q = qs*512 + c*64 + i) for batch 0
    # (rows 0-63) and batch 1 (rows 64-127)
    out = np.empty((B, T, D), np.float32)
    for c in range(NCORES):
        o = np.asarray(results[c]["out"], np.float32)
        for qs in range(QS):
            q0 = qs * SP + c * QB
            out[0, q0:q0 + QB] = o[qs, 0:QB]
            out[1, q0:q0 + QB] = o[qs, QB:2 * QB]
    return out
